# revision 32
# baseline (speedup 1.0000x reference)
"""Bahdanau-style additive attention kernel for Trainium2, 8-core data-parallel.

Math per batch b:
  pq = q_b @ Wq.T            (32, 1024)
  pk = k_b @ Wk.T            (128, 1024)
  v  = linear_att / ||linear_att|| * normalize_scalar
  scores[q,k] = sum_n v[n] * tanh(pq[q,n] + pk[k,n] + bias[n])
  attn = softmax_k(scores);  ctx = attn @ k_b

Device strategy (per core, 4 batches):
  - layout A: n on partitions (8 chunks of 128), (q,k) pairs on free axis
  - MM1/MM2 on PE in bf16 -> pqT [n,(b,q)], pkT [n,(b,k)] (bias folded into
    pqT evac); weights DMA'd n-chunk-major so PE pipelines behind the loads
  - broadcast-sum pq+pk: q-rows [0,8) per chunk via PE identity-matmul
    accumulation into PSUM; rows [8,32) via one DVE tensor_add per chunk-PAIR
    with stride-0 broadcast APs (bf16)
  - tanh on ACT (the hard floor: 16.8M activations/core at 1/cycle/lane),
    one PSUM-sourced + one paired SBUF-sourced activation per chunk, out bf16
  - reduce over n via PE matmuls with stationary v (bf16), psum-accumulated
    into [1,512] blocks packed 4-per-bank at partitions 0/32/64/96
  - scores evac: DVE full-tile copies + strided SBUF->SBUF DMAs -> [32,128]
  - softmax without max-subtraction (|scores| < ~6, exp is safe in fp32);
    single ACT table set (exp_and_others) pinned to avoid table thrashing;
    normalization folded into ctx evac (DVE tensor_scalar_mul)
  - context matmul on PE fp32 (attn kept fp32 for precision); per-batch DMA out
  Engine balance per core: ACT ~141us busy, DVE ~134us, PE ~110us union;
  measured ~176us/core end-to-end incl ~20us DMA-bound startup + ~22us tail.
"""

import numpy as np
import ml_dtypes

import concourse.bass as bass
import concourse.tile as tile
import concourse.mybir as mybir
from concourse import bacc
from concourse.bass_utils import run_bass_kernel_spmd
from concourse.masks import make_identity

P = 128
TQ, TK, B, N = 32, 128, 32, 1024
NCORES = 8
NB = B // NCORES          # batches per core
NCH = N // P              # n chunks
ECH = N // P              # e (contraction) chunks
F32 = mybir.dt.float32
BF16 = mybir.dt.bfloat16
F32R = mybir.dt.float32r
BF = ml_dtypes.bfloat16

# tuning knobs
PE_SUM_BLOCKS = 0     # how many of the 8 512-col blocks per (b,chunk) PE computes
GP_Q = 0
PE_Q = 8              # q-rows per chunk computed by PE identity-matmuls (rest on DVE)
SUM_DT = BF16         # dtype of the pre-tanh sum tile
CTX_F32R = False      # context matmul in float32r (1c/col) instead of fp32 (4c/col)

_CACHE = {}


def _patch_act_tables():
    # Force a single ACT table set containing both Tanh and Exp so the
    # softmax exp never triggers a mid-kernel ACT_TABLE_LOAD (~1.65us each).
    import concourse.bacc as _bacc_mod
    from concourse.hw_specs import get_activation_tables as _gat
    full = _gat("gen3")
    only = {"exp_and_others": full["exp_and_others"]}
    _bacc_mod.get_activation_tables = lambda arch: only


def _build():
    _patch_act_tables()
    nc = bacc.Bacc("TRN2", target_bir_lowering=False, debug=False, num_devices=NCORES)

    wq_d = nc.dram_tensor("wq", [P, NCH, ECH, P], BF16, kind="ExternalInput")  # [ei,nch,ec,nj]
    wk_d = nc.dram_tensor("wk", [P, NCH, ECH, P], BF16, kind="ExternalInput")
    qt_d = nc.dram_tensor("qt", [P, ECH, NB * TQ], BF16, kind="ExternalInput")   # qT[ei,ec,(b,q)]
    kt_d = nc.dram_tensor("kt", [P, ECH, NB * TK], BF16, kind="ExternalInput")   # kT[ei,ec,(b,k)]
    knat_d = nc.dram_tensor("knat", [TK, NB, N], F32, kind="ExternalInput")      # keys[k,b,n]
    v_d = nc.dram_tensor("v2d", [P, NCH], BF16, kind="ExternalInput")            # v[p,ch]
    b_d = nc.dram_tensor("b2d", [P, NCH], F32, kind="ExternalInput")             # bias[p,ch]
    ctx_d = nc.dram_tensor("ctx", [NB, TQ, N], F32, kind="ExternalOutput")

    Tanh = mybir.ActivationFunctionType.Tanh
    Exp = mybir.ActivationFunctionType.Exp
    Ident = mybir.ActivationFunctionType.Identity

    from contextlib import ExitStack
    with tile.TileContext(nc) as tc, ExitStack() as es:
        singles = es.enter_context(tc.tile_pool(name="singles", bufs=1))
        mm_psum = es.enter_context(tc.tile_pool(name="mm_psum", bufs=2, space="PSUM"))
        sc_psum = es.enter_context(tc.tile_pool(name="sc_psum", bufs=2, space="PSUM"))
        pe_psum = es.enter_context(tc.tile_pool(name="pe_psum", bufs=2, space="PSUM"))
        sums = es.enter_context(tc.tile_pool(name="sums", bufs=3))
        tanhs = es.enter_context(tc.tile_pool(name="tanhs", bufs=3))
        small = es.enter_context(tc.tile_pool(name="small", bufs=4))

        # ---- load constants / inputs ----
        wq_sb = singles.tile([P, NCH, ECH, P], BF16)
        wk_sb = singles.tile([P, NCH, ECH, P], BF16)
        qt_sb = singles.tile([P, ECH, NB * TQ], BF16)
        kt_sb = singles.tile([P, ECH, NB * TK], BF16)
        knat_sb = singles.tile([TK, NB, N], F32)
        v_sb = singles.tile([P, NCH], BF16)
        b_sb = singles.tile([P, NCH], F32)
        nc.sync.dma_start(out=qt_sb[:], in_=qt_d[:])
        nc.scalar.dma_start(out=kt_sb[:], in_=kt_d[:])
        for c in range(NCH):
            nc.sync.dma_start(out=wq_sb[:, c], in_=wq_d[:, c])
            nc.scalar.dma_start(out=wk_sb[:, c], in_=wk_d[:, c])
        nc.gpsimd.dma_start(out=v_sb[:], in_=v_d[:])
        nc.gpsimd.dma_start(out=b_sb[:], in_=b_d[:])
        nc.gpsimd.dma_start(out=knat_sb[:], in_=knat_d[:])

        knat_r = singles.tile([TK, NB, N], F32R)
        nc.gpsimd.tensor_copy(out=knat_r[:], in_=knat_sb[:])
        ident32 = singles.tile([TQ, TQ], F32)
        make_identity(nc, ident32[:])
        identb = singles.tile([P, P], BF16)
        make_identity(nc, identb[:])

        # ---- MM1: pqT[n,(b,q)] and MM2: pkT[n,(b,k)] ----
        pqT = singles.tile([P, NCH, NB * TQ], BF16)
        pkT = singles.tile([P, NCH, NB * TK], BF16)
        for nch in range(NCH):
            ps = mm_psum.tile([P, NB * TK], F32, tag="mmps", name=f"pqps{nch}")
            for ech in range(ECH):
                nc.tensor.matmul(
                    ps[:, : NB * TQ],
                    lhsT=wq_sb[:, nch, ech, :],
                    rhs=qt_sb[:, ech, :],
                    start=(ech == 0), stop=(ech == ECH - 1),
                )
            # evacuate with normalize_bias folded in (per-partition bias add)
            nc.vector.tensor_scalar_add(
                out=pqT[:, nch, :], in0=ps[:, : NB * TQ],
                scalar1=b_sb[:, nch:nch + 1],
            )
            ps2 = mm_psum.tile([P, NB * TK], F32, tag="mmps", name=f"pkps{nch}")
            for ech in range(ECH):
                nc.tensor.matmul(
                    ps2[:],
                    lhsT=wk_sb[:, nch, ech, :],
                    rhs=kt_sb[:, ech, :],
                    start=(ech == 0), stop=(ech == ECH - 1),
                )
            nc.vector.tensor_copy(out=pkT[:, nch, :], in_=ps2[:])

        # ---- big loop over batches ----
        for b in range(NB):
            # scores psum: 8 blocks of [1,512] packed 4-per-bank (partitions 0/32/64/96)
            sc_tiles = [sc_psum.tile([P, 512], F32, tag="scps", name=f"scps{b}_{t}") for t in range(2)]
            for pp in range(NCH // 2):
                tanh_t = tanhs.tile([P, 2, TQ, TK], BF16, tag="tanh")
                sum_t = sums.tile([P, 2, TQ - PE_Q, TK], SUM_DT, tag="sum")
                for h in range(2):
                    nch = 2 * pp + h
                    pq_sl = pqT[:, nch, b * TQ:(b + 1) * TQ]
                    pk_sl = pkT[:, nch, b * TK:(b + 1) * TK]
                    # PE computes sum rows [0, PE_Q) via identity-matmul accumulation
                    pe_ps = pe_psum.tile([P, PE_Q, TK], F32, tag="peps",
                                         name=f"peps{b}_{pp}_{h}")
                    for hh in range(PE_Q // 4):
                        nc.tensor.matmul(
                            pe_ps[:, 4 * hh:4 * hh + 4, :],
                            lhsT=identb[:],
                            rhs=pk_sl[:, None, :].to_broadcast([P, 4, TK]),
                            start=True, stop=False, skip_group_check=True,
                        )
                        nc.tensor.matmul(
                            pe_ps[:, 4 * hh:4 * hh + 4, :],
                            lhsT=identb[:],
                            rhs=pq_sl[:, 4 * hh:4 * hh + 4, None].to_broadcast([P, 4, TK]),
                            start=False, stop=True, skip_group_check=True,
                        )
                    nc.scalar.activation(out=tanh_t[:, h, :PE_Q, :], in_=pe_ps[:], func=Tanh)
                # one DVE broadcast-add + one tanh for both chunks' DVE rows
                pq_pair = pqT[:, 2 * pp:2 * pp + 2, b * TQ:(b + 1) * TQ]
                pk_pair = pkT[:, 2 * pp:2 * pp + 2, b * TK:(b + 1) * TK]
                nq = TQ - PE_Q - GP_Q
                nc.vector.tensor_add(
                    out=sum_t[:, :, :nq, :],
                    in0=pq_pair[:, :, PE_Q:PE_Q + nq, None].to_broadcast([P, 2, nq, TK]),
                    in1=pk_pair[:, :, None, :].to_broadcast([P, 2, nq, TK]),
                )
                if GP_Q > 0:
                    nc.gpsimd.tensor_add(
                        out=sum_t[:, :, nq:, :],
                        in0=pq_pair[:, :, PE_Q + nq:, None].to_broadcast([P, 2, GP_Q, TK]),
                        in1=pk_pair[:, :, None, :].to_broadcast([P, 2, GP_Q, TK]),
                    )
                nc.scalar.activation(out=tanh_t[:, :, PE_Q:, :], in_=sum_t[:], func=Tanh)
                for h in range(2):
                    nch = 2 * pp + h
                    for blk in range(8):
                        t, jj = blk // 4, blk % 4
                        nc.tensor.matmul(
                            sc_tiles[t][32 * jj:32 * jj + 1, :],
                            lhsT=v_sb[:, nch:nch + 1],
                            rhs=tanh_t[:, h, 4 * blk:4 * blk + 4, :],
                            start=(nch == 0), stop=(nch == NCH - 1),
                            tile_position=(0, 32 * jj),
                            skip_group_check=True,
                        )
            # evacuate scores: full-tile DVE copies, then strided SBUF->SBUF DMAs
            sc_sb = [small.tile([P, 512], F32, tag="scsb", name=f"scsb{b}_{t}") for t in range(2)]
            for t in range(2):
                nc.vector.tensor_copy(out=sc_sb[t][:], in_=sc_tiles[t][:])
            scores = small.tile([TQ, TK], F32, tag="scores")
            for t in range(2):
                for dq in range(4):
                    src = sc_sb[t][0:P:32, dq * TK:(dq + 1) * TK]          # [4,128]
                    dst = scores[16 * t + dq: 16 * t + dq + 13: 4, :]      # q=16t+4j+dq
                    eng = nc.sync if dq % 2 == 0 else nc.gpsimd
                    eng.dma_start(out=dst, in_=src)
            # softmax (no max subtraction; |scores| is small) + denominator
            attn = small.tile([TQ, TK], F32, tag="attn")
            denom = small.tile([TQ, 1], F32, tag="denom")
            nc.scalar.activation(out=attn[:], in_=scores[:], func=Exp)
            nc.vector.reduce_sum(out=denom[:], in_=attn[:], axis=mybir.AxisListType.X)
            recip = small.tile([TQ, 1], F32, tag="recip")
            nc.vector.reciprocal(out=recip[:], in_=denom[:])
            # attn.T via PE transpose (reuses an mm_psum slot)
            attnT_ps = mm_psum.tile([TK, 512], F32, tag="mmps", name=f"attnT{b}")
            nc.tensor.transpose(attnT_ps[:, :TQ], attn[:], ident32[:])
            attnT = small.tile([TK, TQ], F32R, tag="attnT_sb")
            nc.vector.tensor_copy(out=attnT[:], in_=attnT_ps[:, :TQ])
            # context matmul (two 512-wide halves, each in its own psum slot)
            ctx_sb = small.tile([TQ, N], F32, tag="ctxsb")
            for h in range(2):
                cps = mm_psum.tile([TQ, 512], F32, tag="mmps", name=f"ctxps{b}_{h}")
                nc.tensor.matmul(cps[:], lhsT=attnT[:],
                                 rhs=knat_r[:, b, h * 512:(h + 1) * 512],
                                 start=True, stop=True)
                nc.vector.tensor_scalar_mul(ctx_sb[:, h * 512:(h + 1) * 512], cps[:], recip[:])
            nc.sync.dma_start(out=ctx_d[b], in_=ctx_sb[:])

    nc.compile()
    return nc


def _get_nc():
    key = (PE_SUM_BLOCKS, CTX_F32R)
    if key not in _CACHE:
        _CACHE[key] = _build()
    return _CACHE[key]


LAST_RESULTS = None


def kernel(**inputs) -> tuple:
    global LAST_RESULTS
    query = np.asarray(inputs["query"], np.float32)
    keys = np.asarray(inputs["keys"], np.float32)
    Wq = np.asarray(inputs["Wq"], np.float32)
    Wk = np.asarray(inputs["Wk"], np.float32)
    la = np.asarray(inputs["linear_att"], np.float32)
    nsc = np.asarray(inputs["normalize_scalar"], np.float32)
    nbi = np.asarray(inputs["normalize_bias"], np.float32)

    v = (la / np.linalg.norm(la)) * nsc[0]

    wq_dev = np.ascontiguousarray(
        Wq.T.reshape(ECH, P, NCH, P).transpose(1, 2, 0, 3)).astype(BF)
    wk_dev = np.ascontiguousarray(
        Wk.T.reshape(ECH, P, NCH, P).transpose(1, 2, 0, 3)).astype(BF)
    v_dev = np.ascontiguousarray(v.reshape(NCH, P).T).astype(BF)
    b_dev = np.ascontiguousarray(nbi.reshape(NCH, P).T).astype(np.float32)

    in_maps = []
    for c in range(NCORES):
        b0 = c * NB
        qc = query[:, b0:b0 + NB, :]                   # [tq, nb, n]
        kc = keys[:, b0:b0 + NB, :]                    # [tk, nb, n]
        qt = qc.transpose(2, 1, 0).reshape(ECH, P, NB, TQ).transpose(1, 0, 2, 3)
        qt = np.ascontiguousarray(qt.reshape(P, ECH, NB * TQ)).astype(BF)
        kt = kc.transpose(2, 1, 0).reshape(ECH, P, NB, TK).transpose(1, 0, 2, 3)
        kt = np.ascontiguousarray(kt.reshape(P, ECH, NB * TK)).astype(BF)
        knat = np.ascontiguousarray(kc)                # [tk, nb, n], k on partitions
        in_maps.append({
            "wq": wq_dev, "wk": wk_dev, "qt": qt, "kt": kt,
            "knat": knat, "v2d": v_dev, "b2d": b_dev,
        })

    nc = _get_nc()
    res = run_bass_kernel_spmd(nc, in_maps, core_ids=list(range(NCORES)))
    LAST_RESULTS = res

    context = np.empty((TQ, B, N), np.float32)
    for c in range(NCORES):
        b0 = c * NB
        ctx = res.results[c]["ctx"]                    # [nb, tq, n]
        context[:, b0:b0 + NB, :] = ctx.transpose(1, 0, 2)

    return (query, context, np.concatenate((query, context), axis=2))


# revision 33
# speedup vs baseline: 1.0440x; 1.0440x over previous
"""Bahdanau-style additive attention kernel for Trainium2, 8-core data-parallel.

Math per batch b:
  pq = q_b @ Wq.T            (32, 1024)
  pk = k_b @ Wk.T            (128, 1024)
  v  = linear_att / ||linear_att|| * normalize_scalar
  scores[q,k] = sum_n v[n] * tanh(pq[q,n] + pk[k,n] + bias[n])
  attn = softmax_k(scores);  ctx = attn @ k_b

Device strategy (per core, 4 batches):
  - layout A: n on partitions (8 chunks of 128), (q,k) pairs on free axis
  - MM1/MM2 on PE in bf16 -> pqT [n,(b,q)], pkT [n,(b,k)] (bias folded into
    pqT evac); weights DMA'd n-chunk-major so PE pipelines behind the loads
  - broadcast-sum pq+pk: q-rows [0,8) per chunk via PE identity-matmul
    accumulation into PSUM; rows [8,32) via one DVE tensor_add per chunk-PAIR
    with stride-0 broadcast APs (bf16)
  - tanh on ACT (the hard floor: 16.8M activations/core at 1/cycle/lane),
    one PSUM-sourced + one paired SBUF-sourced activation per chunk, out bf16
  - reduce over n via PE matmuls with stationary v (bf16), psum-accumulated
    into [1,512] blocks packed 4-per-bank at partitions 0/32/64/96
  - scores evac: DVE full-tile copies + strided SBUF->SBUF DMAs -> [32,128]
  - softmax without max-subtraction (|scores| < ~6, exp is safe in fp32);
    single ACT table set (exp_and_others) pinned to avoid table thrashing;
    normalization folded into ctx evac (DVE tensor_scalar_mul)
  - context matmul on PE fp32 (attn kept fp32 for precision); per-batch DMA out
  Engine balance per core: ACT ~141us busy, DVE ~134us, PE ~110us union;
  measured ~176us/core end-to-end incl ~20us DMA-bound startup + ~22us tail.
"""

import numpy as np
import ml_dtypes

import concourse.bass as bass
import concourse.tile as tile
import concourse.mybir as mybir
from concourse import bacc
from concourse.bass_utils import run_bass_kernel_spmd
from concourse.masks import make_identity

P = 128
TQ, TK, B, N = 32, 128, 32, 1024
NCORES = 8
NB = B // NCORES          # batches per core
NCH = N // P              # n chunks
ECH = N // P              # e (contraction) chunks
F32 = mybir.dt.float32
BF16 = mybir.dt.bfloat16
F32R = mybir.dt.float32r
BF = ml_dtypes.bfloat16

# tuning knobs
PE_SUM_BLOCKS = 0     # how many of the 8 512-col blocks per (b,chunk) PE computes
GP_Q = 0
PE_Q = 8              # q-rows per chunk computed by PE identity-matmuls (rest on DVE)
SUM_DT = BF16         # dtype of the pre-tanh sum tile
CTX_F32R = False      # context matmul in float32r (1c/col) instead of fp32 (4c/col)

_CACHE = {}


def _patch_act_tables():
    # Force a single ACT table set containing both Tanh and Exp so the
    # softmax exp never triggers a mid-kernel ACT_TABLE_LOAD (~1.65us each).
    import concourse.bacc as _bacc_mod
    from concourse.hw_specs import get_activation_tables as _gat
    full = _gat("gen3")
    only = {"exp_and_others": full["exp_and_others"]}
    _bacc_mod.get_activation_tables = lambda arch: only


def _build():
    _patch_act_tables()
    nc = bacc.Bacc("TRN2", target_bir_lowering=False, debug=False, num_devices=NCORES)

    wq_d = nc.dram_tensor("wq", [P, NCH, ECH, P], BF16, kind="ExternalInput")  # [ei,nch,ec,nj]
    wk_d = nc.dram_tensor("wk", [P, NCH, ECH, P], BF16, kind="ExternalInput")
    qt_d = nc.dram_tensor("qt", [P, ECH, NB * TQ], BF16, kind="ExternalInput")   # qT[ei,ec,(b,q)]
    kt_d = nc.dram_tensor("kt", [P, ECH, NB * TK], BF16, kind="ExternalInput")   # kT[ei,ec,(b,k)]
    knat_d = nc.dram_tensor("knat", [TK, NB, N], F32, kind="ExternalInput")      # keys[k,b,n]
    v_d = nc.dram_tensor("v2d", [P, NCH], BF16, kind="ExternalInput")            # v[p,ch]
    b_d = nc.dram_tensor("b2d", [P, NCH], F32, kind="ExternalInput")             # bias[p,ch]
    ctx_d = nc.dram_tensor("ctx", [NB, TQ, N], F32, kind="ExternalOutput")

    Tanh = mybir.ActivationFunctionType.Tanh
    Exp = mybir.ActivationFunctionType.Exp
    Ident = mybir.ActivationFunctionType.Identity

    from contextlib import ExitStack
    with tile.TileContext(nc) as tc, ExitStack() as es:
        singles = es.enter_context(tc.tile_pool(name="singles", bufs=1))
        mm_psum = es.enter_context(tc.tile_pool(name="mm_psum", bufs=2, space="PSUM"))
        sc_psum = es.enter_context(tc.tile_pool(name="sc_psum", bufs=2, space="PSUM"))
        pe_psum = es.enter_context(tc.tile_pool(name="pe_psum", bufs=2, space="PSUM"))
        sums = es.enter_context(tc.tile_pool(name="sums", bufs=3))
        tanhs = es.enter_context(tc.tile_pool(name="tanhs", bufs=4))
        small = es.enter_context(tc.tile_pool(name="small", bufs=4))

        # ---- load constants / inputs ----
        wq_sb = singles.tile([P, NCH, ECH, P], BF16)
        wk_sb = singles.tile([P, NCH, ECH, P], BF16)
        qt_sb = singles.tile([P, ECH, NB * TQ], BF16)
        kt_sb = singles.tile([P, ECH, NB * TK], BF16)
        knat_sb = singles.tile([TK, NB, N], F32)
        v_sb = singles.tile([P, NCH], BF16)
        b_sb = singles.tile([P, NCH], F32)
        nc.sync.dma_start(out=qt_sb[:], in_=qt_d[:])
        nc.scalar.dma_start(out=kt_sb[:], in_=kt_d[:])
        for c in range(NCH):
            nc.sync.dma_start(out=wq_sb[:, c], in_=wq_d[:, c])
            nc.scalar.dma_start(out=wk_sb[:, c], in_=wk_d[:, c])
        nc.gpsimd.dma_start(out=v_sb[:], in_=v_d[:])
        nc.gpsimd.dma_start(out=b_sb[:], in_=b_d[:])
        nc.gpsimd.dma_start(out=knat_sb[:], in_=knat_d[:])

        ident32 = singles.tile([TQ, TQ], F32)
        make_identity(nc, ident32[:])
        identb = singles.tile([P, P], BF16)
        make_identity(nc, identb[:])

        # ---- MM1: pqT[n,(b,q)] and MM2: pkT[n,(b,k)] ----
        pqT = singles.tile([P, NCH, NB * TQ], BF16)
        pkT = singles.tile([P, NCH, NB * TK], BF16)
        for nch in range(NCH):
            ps = mm_psum.tile([P, NB * TK], F32, tag="mmps", name=f"pqps{nch}")
            for ech in range(ECH):
                nc.tensor.matmul(
                    ps[:, : NB * TQ],
                    lhsT=wq_sb[:, nch, ech, :],
                    rhs=qt_sb[:, ech, :],
                    start=(ech == 0), stop=(ech == ECH - 1),
                )
            # evacuate with normalize_bias folded in (per-partition bias add)
            nc.vector.tensor_scalar_add(
                out=pqT[:, nch, :], in0=ps[:, : NB * TQ],
                scalar1=b_sb[:, nch:nch + 1],
            )
            ps2 = mm_psum.tile([P, NB * TK], F32, tag="mmps", name=f"pkps{nch}")
            for ech in range(ECH):
                nc.tensor.matmul(
                    ps2[:],
                    lhsT=wk_sb[:, nch, ech, :],
                    rhs=kt_sb[:, ech, :],
                    start=(ech == 0), stop=(ech == ECH - 1),
                )
            nc.vector.tensor_copy(out=pkT[:, nch, :], in_=ps2[:])

        # ---- big loop over batches ----
        for b in range(NB):
            # scores psum: 8 blocks of [1,512] packed 4-per-bank (partitions 0/32/64/96)
            sc_tiles = [sc_psum.tile([P, 512], F32, tag="scps", name=f"scps{b}_{t}") for t in range(2)]
            for pp in range(NCH // 2):
                tanh_t = tanhs.tile([P, 2, TQ, TK], BF16, tag="tanh")
                sum_t = sums.tile([P, 2, TQ - PE_Q, TK], SUM_DT, tag="sum")
                for h in range(2):
                    nch = 2 * pp + h
                    pq_sl = pqT[:, nch, b * TQ:(b + 1) * TQ]
                    pk_sl = pkT[:, nch, b * TK:(b + 1) * TK]
                    # PE computes sum rows [0, PE_Q) via identity-matmul accumulation
                    pe_ps = pe_psum.tile([P, PE_Q, TK], F32, tag="peps",
                                         name=f"peps{b}_{pp}_{h}")
                    for hh in range(PE_Q // 4):
                        nc.tensor.matmul(
                            pe_ps[:, 4 * hh:4 * hh + 4, :],
                            lhsT=identb[:],
                            rhs=pk_sl[:, None, :].to_broadcast([P, 4, TK]),
                            start=True, stop=False, skip_group_check=True,
                        )
                        nc.tensor.matmul(
                            pe_ps[:, 4 * hh:4 * hh + 4, :],
                            lhsT=identb[:],
                            rhs=pq_sl[:, 4 * hh:4 * hh + 4, None].to_broadcast([P, 4, TK]),
                            start=False, stop=True, skip_group_check=True,
                        )
                    nc.scalar.activation(out=tanh_t[:, h, :PE_Q, :], in_=pe_ps[:], func=Tanh)
                # one DVE broadcast-add + one tanh for both chunks' DVE rows
                pq_pair = pqT[:, 2 * pp:2 * pp + 2, b * TQ:(b + 1) * TQ]
                pk_pair = pkT[:, 2 * pp:2 * pp + 2, b * TK:(b + 1) * TK]
                nq = TQ - PE_Q - GP_Q
                nc.vector.tensor_add(
                    out=sum_t[:, :, :nq, :],
                    in0=pq_pair[:, :, PE_Q:PE_Q + nq, None].to_broadcast([P, 2, nq, TK]),
                    in1=pk_pair[:, :, None, :].to_broadcast([P, 2, nq, TK]),
                )
                if GP_Q > 0:
                    nc.gpsimd.tensor_add(
                        out=sum_t[:, :, nq:, :],
                        in0=pq_pair[:, :, PE_Q + nq:, None].to_broadcast([P, 2, GP_Q, TK]),
                        in1=pk_pair[:, :, None, :].to_broadcast([P, 2, GP_Q, TK]),
                    )
                nc.scalar.activation(out=tanh_t[:, :, PE_Q:, :], in_=sum_t[:], func=Tanh)
                for h in range(2):
                    nch = 2 * pp + h
                    for blk in range(8):
                        t, jj = blk // 4, blk % 4
                        nc.tensor.matmul(
                            sc_tiles[t][32 * jj:32 * jj + 1, :],
                            lhsT=v_sb[:, nch:nch + 1],
                            rhs=tanh_t[:, h, 4 * blk:4 * blk + 4, :],
                            start=(nch == 0), stop=(nch == NCH - 1),
                            tile_position=(0, 32 * jj),
                            skip_group_check=True,
                        )
            # evacuate scores: full-tile DVE copies, then strided SBUF->SBUF DMAs
            sc_sb = [small.tile([P, 512], F32, tag="scsb", name=f"scsb{b}_{t}") for t in range(2)]
            for t in range(2):
                nc.vector.tensor_copy(out=sc_sb[t][:], in_=sc_tiles[t][:])
            scores = small.tile([TQ, TK], F32, tag="scores")
            for t in range(2):
                for dq in range(4):
                    src = sc_sb[t][0:P:32, dq * TK:(dq + 1) * TK]          # [4,128]
                    dst = scores[16 * t + dq: 16 * t + dq + 13: 4, :]      # q=16t+4j+dq
                    eng = nc.sync if dq % 2 == 0 else nc.gpsimd
                    eng.dma_start(out=dst, in_=src)
            # softmax (no max subtraction; |scores| is small) + denominator
            attn = small.tile([TQ, TK], F32, tag="attn")
            denom = small.tile([TQ, 1], F32, tag="denom")
            nc.scalar.activation(out=attn[:], in_=scores[:], func=Exp)
            nc.vector.reduce_sum(out=denom[:], in_=attn[:], axis=mybir.AxisListType.X)
            recip = small.tile([TQ, 1], F32, tag="recip")
            nc.vector.reciprocal(out=recip[:], in_=denom[:])
            # attn.T via PE transpose (reuses an mm_psum slot)
            attnT_ps = mm_psum.tile([TK, 512], F32, tag="mmps", name=f"attnT{b}")
            nc.tensor.transpose(attnT_ps[:, :TQ], attn[:], ident32[:])
            attnT = small.tile([TK, TQ], F32, tag="attnT_sb")
            nc.vector.tensor_copy(out=attnT[:], in_=attnT_ps[:, :TQ])
            # context matmul (two 512-wide halves, each in its own psum slot)
            ctx_sb = small.tile([TQ, N], F32, tag="ctxsb")
            for h in range(2):
                cps = mm_psum.tile([TQ, 512], F32, tag="mmps", name=f"ctxps{b}_{h}")
                nc.tensor.matmul(cps[:], lhsT=attnT[:],
                                 rhs=knat_sb[:, b, h * 512:(h + 1) * 512],
                                 start=True, stop=True)
                nc.vector.tensor_scalar_mul(ctx_sb[:, h * 512:(h + 1) * 512], cps[:], recip[:])
            nc.sync.dma_start(out=ctx_d[b], in_=ctx_sb[:])

    nc.compile()
    return nc


def _get_nc():
    key = (PE_SUM_BLOCKS, CTX_F32R)
    if key not in _CACHE:
        _CACHE[key] = _build()
    return _CACHE[key]


LAST_RESULTS = None


def kernel(**inputs) -> tuple:
    global LAST_RESULTS
    query = np.asarray(inputs["query"], np.float32)
    keys = np.asarray(inputs["keys"], np.float32)
    Wq = np.asarray(inputs["Wq"], np.float32)
    Wk = np.asarray(inputs["Wk"], np.float32)
    la = np.asarray(inputs["linear_att"], np.float32)
    nsc = np.asarray(inputs["normalize_scalar"], np.float32)
    nbi = np.asarray(inputs["normalize_bias"], np.float32)

    v = (la / np.linalg.norm(la)) * nsc[0]

    wq_dev = np.ascontiguousarray(
        Wq.T.reshape(ECH, P, NCH, P).transpose(1, 2, 0, 3)).astype(BF)
    wk_dev = np.ascontiguousarray(
        Wk.T.reshape(ECH, P, NCH, P).transpose(1, 2, 0, 3)).astype(BF)
    v_dev = np.ascontiguousarray(v.reshape(NCH, P).T).astype(BF)
    b_dev = np.ascontiguousarray(nbi.reshape(NCH, P).T).astype(np.float32)

    in_maps = []
    for c in range(NCORES):
        b0 = c * NB
        qc = query[:, b0:b0 + NB, :]                   # [tq, nb, n]
        kc = keys[:, b0:b0 + NB, :]                    # [tk, nb, n]
        qt = qc.transpose(2, 1, 0).reshape(ECH, P, NB, TQ).transpose(1, 0, 2, 3)
        qt = np.ascontiguousarray(qt.reshape(P, ECH, NB * TQ)).astype(BF)
        kt = kc.transpose(2, 1, 0).reshape(ECH, P, NB, TK).transpose(1, 0, 2, 3)
        kt = np.ascontiguousarray(kt.reshape(P, ECH, NB * TK)).astype(BF)
        knat = np.ascontiguousarray(kc)                # [tk, nb, n], k on partitions
        in_maps.append({
            "wq": wq_dev, "wk": wk_dev, "qt": qt, "kt": kt,
            "knat": knat, "v2d": v_dev, "b2d": b_dev,
        })

    nc = _get_nc()
    res = run_bass_kernel_spmd(nc, in_maps, core_ids=list(range(NCORES)))
    LAST_RESULTS = res

    context = np.empty((TQ, B, N), np.float32)
    for c in range(NCORES):
        b0 = c * NB
        ctx = res.results[c]["ctx"]                    # [nb, tq, n]
        context[:, b0:b0 + NB, :] = ctx.transpose(1, 0, 2)

    return (query, context, np.concatenate((query, context), axis=2))
